# revision 21
# baseline (speedup 1.0000x reference)
"""Trainium2 Bass kernel for nn_Beta_LR_41308995453190.

Network (per (b, o) pair):
  - 13 segment means over the L axis of hidden[b, o] (ragged boundaries
    from idx[b]): 10 context segments, question, option, whole-context.
  - beta-param projection e = 1 + relu(x @ Wp + bp), split a/b.
  - three attention pools (intersection over segments, renew over
    (segment, intersection) pairs, union over inverted renewed params).
  - classify head: concat 8 beta embeddings -> relu(@Wl0 + bl0) -> @Wl + bl.

Sharding: data-parallel over the batch dim B=8 (one batch per NeuronCore),
weights replicated.

Implementation notes:
  - Segment sums are 0/1-mask matmuls (mask as the 13-column stationary
    operand, hidden streaming 512 wide), scaled by 1/count afterwards.
    Hidden and mask travel in fp8 e3m4 (the mask is exactly representable,
    products accumulate in fp32 PSUM), halving the dominant DMA stream;
    hidden is laid out l = p*T + t so each partition's read is one
    contiguous 8KB block, and all four options are SBUF-resident so the
    segsum phase never stalls on buffer reuse.
  - The whole MLP stack (e, h1/l1, h2/l2, h3/l3) runs weight-stationary
    and feature-major: the weight 128x128 blocks are the stationary
    operand and the (tiny) activation blocks stream, so nothing is ever
    transposed back and no PSUM->SBUF row copies exist.  Each f-chunk
    group starts with a K=1 "bias row x ones" matmul that initializes
    the accumulator with the per-feature bias, so one batched
    scalar-engine activation (relu / exp) per layer reads PSUM directly
    with no per-chunk bias columns (which would re-serialize the
    groups).  The e-layer computes e-1 = relu(x@Wp + bp) and the +1 is
    absorbed into downstream biases (colsum of Wa0 / rowsum of Wl0)
    and the pair-softmax algebra.  The softmaxes skip max-subtraction
    (logits are 0.02-scale products, bounded far inside exp range) and
    the two pair/union reciprocals are batched into one DVE op each.
  - The classify head streams bf16 Wl0 against the catF column blocks
    into a dedicated PSUM bank in three emission groups interleaved
    with the pool phases, so those matmuls fill the PE gaps under the
    softmax chains; bl0 is added by one extra accumulation matmul
    (ones-column x bl0/128), and the final relu/dot/+bl collapses into
    a tensor_scalar_max plus one fused tensor_tensor_reduce.
"""

import numpy as np
import ml_dtypes

try:
    import concourse.bass as bass
except ImportError:
    import sys

    sys.path.insert(0, "/opt/trn_rl_repo")
    import concourse.bass as bass

import concourse.tile as tile
from concourse import mybir
from concourse.bass_utils import run_bass_kernel_spmd
from concourse.masks import make_identity

F32 = mybir.dt.float32
BF16 = mybir.dt.bfloat16
F8 = mybir.dt.float8e4
NPBF16 = ml_dtypes.bfloat16
NPF8 = ml_dtypes.float8_e4m3
AX = mybir.AxisListType.X
OP = mybir.AluOpType
AF = mybir.ActivationFunctionType

B, O, L, E = 8, 4, 1024, 1024
BETA = 512
NSEG = 12
NK = 13  # 10 ctx + q + o + allc
P = 128
T = L // P  # 8 L-tiles per option
NCOL = O * NK  # 52

HID_FP8 = True  # hidden + mask in fp8 e4m3 (halves hidden DMA); DoubleRow
TD = 4  # L-tile pairs per option in DoubleRow mode

HDT, NPHDT = (F8, NPF8) if HID_FP8 else (BF16, NPBF16)


def _split_excess_waits(nc, max_waits=1):
    """This neuronxcc walrus build rejects more than one sem wait per TPB
    instruction; hoist excess waits onto drain instructions inserted before
    the offending instruction on the same engine."""
    scratch_bb = nc.cur_bb.bb
    for f in nc.m.functions:
        for bb in f.blocks:
            new_list = []
            for ins in bb.instructions:
                si = ins.sync_info
                waits = list(si.on_wait) if si and si.on_wait else []
                if len(waits) > max_waits:
                    for w in waits[: len(waits) - max_waits]:
                        carrier = nc.engines[ins.engine].nop(nofuse=True).ins
                        scratch_bb.instructions.remove(carrier)
                        carrier.sync_info = mybir.SyncInfo(
                            on_wait=[w], on_update=[]
                        )
                        new_list.append(carrier)
                    si.on_wait = waits[len(waits) - max_waits :]
                new_list.append(ins)
            bb.instructions[:] = new_list


def _build_nc(debug=False):
    nc = bass.Bass("TRN2", target_bir_lowering=False)

    hid_d = nc.dram_tensor("hidden", [O, L, E], HDT, kind="ExternalInput")
    mask_d = nc.dram_tensor("maskt", [P, TD, 2, 16], HDT, kind="ExternalInput")
    cnt_d = nc.dram_tensor("cntinv", [NK, 1], F32, kind="ExternalInput")
    wp_d = nc.dram_tensor("wp", [P, 8, 1024], BF16, kind="ExternalInput")
    wa0_d = nc.dram_tensor("wa0", [P, 8, 512], BF16, kind="ExternalInput")
    wa_d = nc.dram_tensor("wa", [P, 4, 512], BF16, kind="ExternalInput")
    wl0_d = nc.dram_tensor("wl0", [P, 32, 512], BF16, kind="ExternalInput")
    bias_d = nc.dram_tensor("biases", [P, 21], F32, kind="ExternalInput")
    brow_d = nc.dram_tensor("biasrow", [1, 20 * P], BF16, kind="ExternalInput")
    bl0r_d = nc.dram_tensor("bl0rep", [P, 512], BF16, kind="ExternalInput")
    wlr_d = nc.dram_tensor("wlrep", [P, 4], F32, kind="ExternalInput")
    out_d = nc.dram_tensor("out", [O, 1], F32, kind="ExternalOutput")

    with tile.TileContext(nc) as tc:
        with (
            tc.tile_pool(name="const", bufs=1) as const,
            tc.tile_pool(name="act", bufs=1) as act,
            tc.tile_pool(name="tmp", bufs=3) as tmp,
            tc.tile_pool(name="rows", bufs=1) as rowsp,
            tc.tile_pool(name="pseg", bufs=2, space="PSUM") as pseg,
            tc.tile_pool(name="pwork", bufs=2, space="PSUM") as pwork,
            tc.tile_pool(name="pf", bufs=1, space="PSUM") as pfp,
            tc.tile_pool(name="pt", bufs=3, space="PSUM") as pt,
        ):
            # ---- DMA issue order: mask, hidden o0 (fine-grained), cnt,
            # hidden o1-3, wp, wa0/wa, biases, wl0 (classify-chunk order).
            # hidden: l = p*T + t layout -> per-partition contiguous reads
            hid_r = hid_d.rearrange("o (p t) e -> o p t e", t=T)
            htile = const.tile([P, O, TD, 2, E], HDT)
            hv = htile.rearrange("p o t j e -> p o (t j e)")
            for h in range(4):
                nc.sync.dma_start(
                    out=hv[:, 0, h * 2048 : h * 2048 + 2048],
                    in_=hid_r[0][:, h * 2 : h * 2 + 2, :].rearrange(
                        "p t e -> p (t e)"
                    ),
                )
            mask_sb = const.tile([P, TD, 2, 16], HDT)
            nc.sync.dma_start(out=mask_sb, in_=mask_d[:])
            for h in range(4):
                nc.sync.dma_start(
                    out=hv[:, 1, h * 2048 : h * 2048 + 2048],
                    in_=hid_r[1][:, h * 2 : h * 2 + 2, :].rearrange(
                        "p t e -> p (t e)"
                    ),
                )
            cnt_sb = const.tile([NK, 1], F32)
            nc.sync.dma_start(out=cnt_sb, in_=cnt_d[:])
            brow_sb = const.tile([1, 20 * P], BF16)
            nc.sync.dma_start(out=brow_sb, in_=brow_d[:])
            bl0r_sb = const.tile([P, 512], BF16)
            nc.sync.dma_start(out=bl0r_sb, in_=bl0r_d[:])
            wlt_sb = const.tile([P, 4], F32)
            nc.sync.dma_start(out=wlt_sb, in_=wlr_d[:])
            for o in range(2, O):
                for h in range(2):
                    nc.sync.dma_start(
                        out=hv[:, o, h * 4096 : h * 4096 + 4096],
                        in_=hid_r[o][:, h * 4 : h * 4 + 4, :].rearrange(
                            "p t e -> p (t e)"
                        ),
                    )
            wp_sb = const.tile([P, 8, 1024], BF16)
            nc.sync.dma_start(out=wp_sb[:, 0:4, :], in_=wp_d[:, 0:4, :])
            nc.sync.dma_start(out=wp_sb[:, 4:8, :], in_=wp_d[:, 4:8, :])
            wa0_sb = const.tile([P, 8, 512], BF16)
            nc.sync.dma_start(out=wa0_sb, in_=wa0_d[:])
            wa_sb = const.tile([P, 4, 512], BF16)
            nc.sync.dma_start(out=wa_sb, in_=wa_d[:])
            bias_sb = const.tile([P, 21], F32)
            nc.sync.dma_start(out=bias_sb, in_=bias_d[:])
            wl0_sb = const.tile([P, 32, 512], BF16)
            for a, b in ((8, 16), (16, 24), (24, 32), (0, 8)):
                nc.sync.dma_start(out=wl0_sb[:, a:b, :], in_=wl0_d[:, a:b, :])

            ident = const.tile([P, P], F32)
            make_identity(nc, ident)
            identb = const.tile([P, P], BF16)
            make_identity(nc, identb)
            ones4 = const.tile([P, O], BF16)
            nc.vector.memset(ones4, 1.0)
            ones1f = const.tile([P, 1], F32)
            nc.vector.memset(ones1f, 1.0)
            ones1r = const.tile([1, NCOL], BF16)
            nc.vector.memset(ones1r, 1.0)

            def brow(i):
                return brow_sb[0:1, i * P : (i + 1) * P]

            def bcol(i):
                return bias_sb[:, i : i + 1]

            # ---- segment sums: ps[k, e] = sum over rows of seg k (0/1 mask)
            # then x = ps * cntinv; x_all packs one 32-aligned row block per
            # option (partition bases must be 32-aligned)
            x_all = rowsp.tile([P, E], BF16, tag="x_all")
            nc.vector.memset(x_all, 0.0)
            DR = mybir.MatmulPerfMode.DoubleRow
            xT = act.tile([P, 8, O, NK], BF16)

            def xt_transpose(c):
                ptile = pt.tile([P, P], BF16, tag="pt")
                nc.tensor.transpose(
                    out=ptile,
                    in_=x_all[:, c * P : (c + 1) * P],
                    identity=identb[:, :],
                )
                if c % 2 == 0:
                    nc.scalar.copy(
                        out=xT[:, c, :, :],
                        in_=ptile.rearrange("p (o k) -> p o k", k=32)[:, :, 0:NK],
                    )
                else:
                    nc.vector.tensor_copy(
                        out=xT[:, c, :, :],
                        in_=ptile.rearrange("p (o k) -> p o k", k=32)[:, :, 0:NK],
                    )

            # half-outer so the first 4 feature transposes overlap the
            # second half's segsum matmuls
            for half in range(2):
                sl = slice(half * 512, half * 512 + 512)
                for o in range(O):
                    ps = pseg.tile([16, 512], F32, tag="ps_seg")
                    for t in range(TD):
                        nc.tensor.matmul(
                            out=ps,
                            lhsT=mask_sb[:, t, :, :],
                            rhs=htile[:, o, t, :, sl],
                            start=(t == 0),
                            stop=(t == TD - 1),
                            perf_mode=DR,
                        )
                    nc.vector.tensor_scalar_mul(
                        out=x_all[o * 32 : o * 32 + NK, sl],
                        in0=ps[0:NK, :],
                        scalar1=cnt_sb[:, :],
                    )
                for c in range(half * 4, half * 4 + 4):
                    xt_transpose(c)

            # ---- projection, weight-stationary: eTb = e-1 = relu(x@Wp+bp)
            # (the +1 is absorbed into downstream biases and the pair
            # algebra); one K=1 bias-row matmul initializes each f-chunk.
            eTb = act.tile([P, 8, NCOL], BF16)
            eps = pwork.tile([P, 8, NCOL], F32, tag="pwork", space="PSUM")
            # bias-init matmuls first (no xT dependence: they run during the
            # xT transpose/copy chain), then c-outer passes chase the xT
            # chunk copies as they land.
            for f in range(8):
                nc.tensor.matmul(
                    out=eps[:, f, :], lhsT=brow(f), rhs=ones1r,
                    start=True, stop=False,
                )
            for c in range(8):
                for f in range(8):
                    nc.tensor.matmul(
                        out=eps[:, f, :],
                        lhsT=wp_sb[:, c, f * P : (f + 1) * P],
                        rhs=xT[:, c, :, :],
                        start=False,
                        stop=(c == 7),
                    )
            nc.vector.tensor_scalar_max(out=eTb, in0=eps, scalar1=0.0)
            eTr = eTb.rearrange("p c (o k) -> p c o k", k=NK)

            # catFb chunks 8..31 (a_ac, b_ac, a_o, b_o, a_q, b_q) only need
            # eTb; filling them now lets the classify-head matmuls over those
            # chunks run inside tensor-engine gaps during the softmax phases.
            catFb = act.tile([P, 32, O], BF16)
            for j, (half, k) in enumerate(
                ((0, 12), (1, 12), (0, 11), (1, 11), (0, 10), (1, 10))
            ):
                nc.gpsimd.tensor_copy(
                    out=catFb[:, 8 + j * 4 : 12 + j * 4, :],
                    in_=eTr[:, half * 4 : half * 4 + 4, :, k],
                )

            # ---- pool 1 (intersection), weight-stationary feature-major:
            # h1 = relu(e @ Wa0 + ba0), only the 40 ctx rows
            h1Tb = act.tile([P, 4, O, 10], BF16)
            h1ps = pwork.tile([P, 4, O * 10], F32, tag="pwork", space="PSUM")
            for f in range(4):
                nc.tensor.matmul(
                    out=h1ps[:, f, :], lhsT=brow(8 + f),
                    rhs=ones1r[0:1, 0:40], start=True, stop=False,
                )
                for c in range(8):
                    nc.tensor.matmul(
                        out=h1ps[:, f, :],
                        lhsT=wa0_sb[:, c, f * P : (f + 1) * P],
                        rhs=eTr[:, c, :, 0:10],
                        start=False,
                        stop=(c == 7),
                    )
            nc.scalar.activation(
                out=h1Tb,
                in_=h1ps.rearrange("p f (o k) -> p f o k", o=O),
                func=AF.Relu,
            )

            # l1 = h1 @ Wa + ba; w = exp(l1) fused from PSUM (reused as the
            # pair-softmax numerator e1 below)
            w = act.tile([P, 4, O, 10], F32)
            l1ps = pwork.tile([P, 4, O * 10], F32, tag="pwork", space="PSUM")
            for f in range(4):
                nc.tensor.matmul(
                    out=l1ps[:, f, :], lhsT=brow(12 + f),
                    rhs=ones1r[0:1, 0:40], start=True, stop=False,
                )
                for c in range(4):
                    nc.tensor.matmul(
                        out=l1ps[:, f, :],
                        lhsT=wa_sb[:, c, f * P : (f + 1) * P],
                        rhs=h1Tb[:, c, :, :],
                        start=False,
                        stop=(c == 3),
                    )
            nc.scalar.activation(
                out=w,
                in_=l1ps.rearrange("p f (o k) -> p f o k", o=O),
                func=AF.Exp,
            )

            # ---- classify head part 1: bl0 (via ones x bl0/128) + chunks
            # 8..19; these run on the PE while the vector engine does the
            # pool-1 softmax below.
            pf = pfp.tile([O, 512], F32, tag="pf")
            nc.tensor.matmul(
                out=pf, lhsT=ones4, rhs=bl0r_sb, start=True, stop=False
            )
            kc_order = list(range(8, 32)) + list(range(8))
            for kc in kc_order[:6]:
                nc.tensor.matmul(
                    out=pf,
                    lhsT=catFb[:, kc, :],
                    rhs=wl0_sb[:, kc, :],
                    start=False,
                    stop=False,
                )

            # pool 1 softmax over the 10 ctx segments + weighted reduce
            # (no max-subtraction; wt_a/wt_b are reused by the pair phase)
            cat2b = act.tile([P, 8, O], BF16)
            s = tmp.tile([P, 4, O], F32, tag="s")
            nc.vector.reduce_sum(s, w, axis=AX)
            r = tmp.tile([P, 4, O], F32, tag="r")
            nc.vector.reciprocal(out=r, in_=s)
            wts = []
            for half in range(2):
                eng = nc.vector if half == 0 else nc.gpsimd
                wt = tmp.tile([P, 4, O, 10], F32, tag=f"wt{half}")
                eng.tensor_tensor(
                    out=wt, in0=w, in1=eTr[:, half * 4 : half * 4 + 4, :, 0:10],
                    op=OP.mult,
                )
                wts.append(wt)
                st = tmp.tile([P, 4, O], F32, tag=f"st{half}")
                nc.vector.reduce_sum(st, wt, axis=AX)
                nc.vector.tensor_tensor(
                    out=cat2b[:, half * 4 : half * 4 + 4, :], in0=st, in1=r,
                    op=OP.mult,
                )

            # ---- renew: h2/l2 for the intersection pair element
            # (weight-stationary, 4 rows)
            h2Tb = act.tile([P, 4, O], BF16)
            h2ps = pwork.tile([P, 4, O], F32, tag="pwork", space="PSUM")
            for f in range(4):
                nc.tensor.matmul(
                    out=h2ps[:, f, :], lhsT=brow(8 + f),
                    rhs=ones1r[0:1, 0:O], start=True, stop=False,
                )
                for c in range(8):
                    nc.tensor.matmul(
                        out=h2ps[:, f, :],
                        lhsT=wa0_sb[:, c, f * P : (f + 1) * P],
                        rhs=cat2b[:, c, :],
                        start=False,
                        stop=(c == 7),
                    )
            nc.scalar.activation(out=h2Tb, in_=h2ps, func=AF.Relu)
            e2 = tmp.tile([P, 4, O], F32, tag="e2")
            l2ps = pwork.tile([P, 4, O], F32, tag="pwork", space="PSUM")
            for f in range(4):
                nc.tensor.matmul(
                    out=l2ps[:, f, :], lhsT=brow(12 + f),
                    rhs=ones1r[0:1, 0:O], start=True, stop=False,
                )
                for c in range(4):
                    nc.tensor.matmul(
                        out=l2ps[:, f, :],
                        lhsT=wa_sb[:, c, f * P : (f + 1) * P],
                        rhs=h2Tb[:, c, :],
                        start=False,
                        stop=(c == 3),
                    )
            nc.scalar.activation(out=e2, in_=l2ps, func=AF.Exp)

            # ---- classify head part 2: chunks 20..31 (run under the pair
            # softmax)
            for kc in kc_order[6:16]:
                nc.tensor.matmul(
                    out=pf,
                    lhsT=catFb[:, kc, :],
                    rhs=wl0_sb[:, kc, :],
                    start=False,
                    stop=False,
                )

            # pair softmax([l1[k], l2]) -> 1/na, 1/nb = s12 / (e1*a + e2*pool)
            raTb = act.tile([P, 4, O, 10], BF16)
            rbTb = act.tile([P, 4, O, 10], BF16)
            s12 = tmp.tile([P, 4, O, 10], F32, tag="s12")
            nc.vector.tensor_tensor(
                out=s12, in0=w, in1=e2.broadcast_to([P, 4, O, 10]), op=OP.add
            )
            t3 = tmp.tile([P, 8, O, 10], F32, tag="t3")
            for half in range(2):
                q = tmp.tile([P, 4, O], F32, tag="q")
                nc.vector.tensor_tensor(
                    out=q, in0=e2, in1=cat2b[:, half * 4 : half * 4 + 4, :],
                    op=OP.mult,
                )
                ta = tmp.tile([P, 4, O, 10], F32, tag="ta")
                nc.vector.tensor_tensor(
                    out=ta, in0=wts[half], in1=s12, op=OP.add
                )
                nc.vector.tensor_tensor(
                    out=t3[:, half * 4 : half * 4 + 4, :, :], in0=ta,
                    in1=q.broadcast_to([P, 4, O, 10]), op=OP.add,
                )
            it3 = tmp.tile([P, 8, O, 10], F32, tag="it3")
            nc.vector.reciprocal(out=it3, in_=t3)
            for half, dstb in ((0, raTb), (1, rbTb)):
                nc.vector.tensor_tensor(
                    out=dstb[:, :, :, :], in0=s12,
                    in1=it3[:, half * 4 : half * 4 + 4, :, :], op=OP.mult
                )

            # ---- union pool (weight-stationary, 40 rows)
            h3Tb = act.tile([P, 4, O, 10], BF16)
            h3ps = pwork.tile([P, 4, O * 10], F32, tag="pwork", space="PSUM")
            for f in range(4):
                nc.tensor.matmul(
                    out=h3ps[:, f, :], lhsT=brow(16 + f),
                    rhs=ones1r[0:1, 0:40], start=True, stop=False,
                )
                for c in range(8):
                    rsrc = raTb if c < 4 else rbTb
                    nc.tensor.matmul(
                        out=h3ps[:, f, :],
                        lhsT=wa0_sb[:, c, f * P : (f + 1) * P],
                        rhs=rsrc[:, c % 4, :, :],
                        start=False,
                        stop=(c == 7),
                    )
            nc.scalar.activation(
                out=h3Tb,
                in_=h3ps.rearrange("p f (o k) -> p f o k", o=O),
                func=AF.Relu,
            )
            w3 = tmp.tile([P, 4, O, 10], F32, tag="w3")
            l3ps = pwork.tile([P, 4, O * 10], F32, tag="pwork", space="PSUM")
            for f in range(4):
                nc.tensor.matmul(
                    out=l3ps[:, f, :], lhsT=brow(12 + f),
                    rhs=ones1r[0:1, 0:40], start=True, stop=False,
                )
                for c in range(4):
                    nc.tensor.matmul(
                        out=l3ps[:, f, :],
                        lhsT=wa_sb[:, c, f * P : (f + 1) * P],
                        rhs=h3Tb[:, c, :, :],
                        start=False,
                        stop=(c == 3),
                    )
            nc.scalar.activation(
                out=w3,
                in_=l3ps.rearrange("p f (o k) -> p f o k", o=O),
                func=AF.Exp,
            )

            # ---- classify head part 3: chunks 26..31 (run under the union
            # softmax)
            for kc in kc_order[16:24]:
                nc.tensor.matmul(
                    out=pf,
                    lhsT=catFb[:, kc, :],
                    rhs=wl0_sb[:, kc, :],
                    start=False,
                    stop=False,
                )

            # union softmax + weighted reduce + invert -> catFb chunks 0..7
            # ua = s3 / (sum_k w3 ra)
            s3 = tmp.tile([P, 4, O], F32, tag="s3")
            nc.vector.reduce_sum(s3, w3, axis=AX)
            su = tmp.tile([P, 8, O], F32, tag="su")
            for half, rsrc in ((0, raTb), (1, rbTb)):
                eng = nc.vector if half == 0 else nc.gpsimd
                tu = tmp.tile([P, 4, O, 10], F32, tag=f"tu{half}")
                eng.tensor_tensor(
                    out=tu, in0=w3, in1=rsrc[:, :, :, :], op=OP.mult
                )
                nc.vector.reduce_sum(
                    su[:, half * 4 : half * 4 + 4, :], tu, axis=AX
                )
            isu = tmp.tile([P, 8, O], F32, tag="isu")
            nc.vector.reciprocal(out=isu, in_=su)
            for half in range(2):
                nc.vector.tensor_tensor(
                    out=catFb[:, half * 4 : half * 4 + 4, :], in0=s3,
                    in1=isu[:, half * 4 : half * 4 + 4, :], op=OP.mult,
                )

            # ---- classify head part 3: chunks 0..7 (ua/ub), close the
            # accumulation
            for i, kc in enumerate(kc_order[24:]):
                nc.tensor.matmul(
                    out=pf,
                    lhsT=catFb[:, kc, :],
                    rhs=wl0_sb[:, kc, :],
                    start=False,
                    stop=(i == 7),
                )
            # out = relu(hf) . Wl + bl, feature-major: relu rides the
            # PSUM->SBUF copy (scalar), then 4 transposes put features on
            # partitions, a per-partition Wl multiply + 4-wide reduce, and a
            # ones-column matmul does the partition sum.
            hrelu = rowsp.tile([O, 512], F32, tag="hrelu")
            nc.scalar.activation(out=hrelu, in_=pf[:, :], func=AF.Relu)
            hwT = rowsp.tile([P, O, 4], F32, tag="hwT")
            for c in range(4):
                ptile = pt.tile([P, O], F32, tag="pt")
                nc.tensor.transpose(
                    out=ptile,
                    in_=hrelu[:, c * P : (c + 1) * P],
                    identity=ident[:O, :O],
                )
                nc.vector.tensor_scalar_mul(
                    out=hwT[:, :, c], in0=ptile, scalar1=wlt_sb[:, c : c + 1]
                )
            sumT = rowsp.tile([P, O], F32, tag="sumT")
            nc.vector.reduce_sum(sumT, hwT, axis=AX)
            res = pt.tile([1, O], F32, tag="pt", space="PSUM")
            nc.tensor.matmul(
                out=res, lhsT=ones1f[:, :], rhs=sumT, start=True, stop=True
            )
            out_sb = rowsp.tile([1, O], F32, tag="out_sb")
            nc.vector.tensor_scalar_add(
                out=out_sb, in0=res, scalar1=bias_sb[0:1, 20:21]
            )
            nc.sync.dma_start(out=out_d.rearrange("o i -> i o"), in_=out_sb)

            if debug:
                for name, t, dt in (
                    ("xT", xT, BF16),
                    ("eTb", eTb, BF16),
                    ("w", w, F32),
                    ("cat2b", cat2b, BF16),
                    ("raTb", raTb, BF16),
                    ("rbTb", rbTb, BF16),
                    ("catFb", catFb, BF16),
                ):
                    d = nc.dram_tensor(
                        "dbg_" + name, list(t.shape), dt, kind="ExternalOutput"
                    )
                    nc.sync.dma_start(out=d[:], in_=t)

    _split_excess_waits(nc)
    return nc


_NC = None


def _get_nc():
    global _NC
    if _NC is None:
        _NC = _build_nc()
    return _NC


def _prep_inputs(hidden, idx, Wp, bp, Wa0, ba0, Wa, ba, Wl0, bl0, Wl, bl):
    hidden = np.asarray(hidden, dtype=np.float32)
    idx = np.asarray(idx).astype(np.int64)

    f32 = lambda a: np.ascontiguousarray(np.asarray(a, dtype=np.float32))
    bf = lambda a: np.ascontiguousarray(np.asarray(a, dtype=np.float32).astype(NPBF16))
    bp, ba0, ba, bl0, bl = f32(bp), f32(ba0), f32(ba), f32(bl0), f32(bl)
    Wl = f32(Wl)

    hid_b = np.ascontiguousarray(hidden.astype(NPHDT))  # [B, O, L, E]
    wp_t = bf(np.asarray(Wp, np.float32).reshape(8, P, 1024).transpose(1, 0, 2))
    wa0_t = bf(np.asarray(Wa0, np.float32).reshape(8, P, 512).transpose(1, 0, 2))
    wa_t = bf(np.asarray(Wa, np.float32).reshape(4, P, 512).transpose(1, 0, 2))
    wl0_t = bf(np.asarray(Wl0, np.float32).reshape(32, P, 512).transpose(1, 0, 2))

    biases = np.zeros((P, 21), dtype=np.float32)
    biases[:, 20] = bl[0]

    # bias row for the K=1 accumulator-init matmuls; the e-layer computes
    # e-1 so h1/h2 absorb the +1 via the column sums of Wa0 and the head
    # absorbs it via the row sums of Wl0[1024:].
    ba0p = ba0 + np.asarray(Wa0, np.float32).sum(axis=0)
    brow = np.zeros((1, 20 * P), dtype=np.float32)
    brow[0, 0 : 8 * P] = bp
    brow[0, 8 * P : 12 * P] = ba0p
    brow[0, 12 * P : 16 * P] = ba
    brow[0, 16 * P : 20 * P] = ba0
    brow = bf(brow)

    bl0p = bl0 + np.asarray(Wl0, np.float32)[1024:4096, :].sum(axis=0)
    bl0rep = bf(np.broadcast_to(bl0p / 128.0, (P, 512)))
    wlrep = np.ascontiguousarray(Wl[:, 0].reshape(4, P).T.astype(np.float32))

    in_maps = []
    for b in range(B):
        m = np.zeros((L, NK), dtype=np.float32)
        cntinv = np.zeros((NK, 1), dtype=np.float32)
        ib = idx[b]
        starts = [1] + [int(ib[k]) for k in range(9)]
        ends = [int(ib[k]) for k in range(10)]
        segs = [(starts[k], ends[k]) for k in range(10)]
        segs.append((int(ib[9]), int(ib[10])))
        segs.append((int(ib[10]), int(ib[11])))
        segs.append((1, int(ib[9])))
        for k, (s, e) in enumerate(segs):
            m[s:e, k] = 1.0
            cntinv[k, 0] = 1.0 / (e - s)
        # l = p*8 + 2t' + j layout, planar pairs padded to [P, 4, 2, 16]
        mp = np.zeros((P, 4, 2, 16), dtype=np.float32)
        mp[:, :, :, 0:NK] = m.reshape(P, 4, 2, NK)
        maskt = np.ascontiguousarray(mp.astype(NPHDT))

        in_maps.append(
            dict(
                hidden=np.ascontiguousarray(hid_b[b]),
                maskt=maskt,
                cntinv=cntinv,
                wp=wp_t,
                wa0=wa0_t,
                wa=wa_t,
                wl0=wl0_t,
                biases=biases,
                biasrow=brow,
                bl0rep=bl0rep,
                wlrep=wlrep,
            )
        )
    return in_maps


def _run(in_maps, **kwargs):
    return run_bass_kernel_spmd(_get_nc(), in_maps, core_ids=list(range(B)), **kwargs)


def kernel(**inputs):
    in_maps = _prep_inputs(**inputs)
    res = _run(in_maps)
    return np.stack([r["out"].reshape(O, 1) for r in res.results])


def _install_ntff_hook():
    """The RL container's antenv lacks axon_hooks, so boot() skipped NTFF
    hook registration. Recreate the module and register the ctypes hook."""
    import sys
    import types

    name = "antenv.axon_hooks"
    if name not in sys.modules:
        try:
            __import__(name)
        except ImportError:
            mod = types.ModuleType(name)
            mod._hook = None
            mod.set_axon_ntff_profile_hook = lambda h: setattr(mod, "_hook", h)
            mod.get_axon_ntff_profile_hook = lambda: mod._hook
            sys.modules[name] = mod
            import antenv

            antenv.axon_hooks = mod
    import antenv.axon_hooks as ah

    if ah.get_axon_ntff_profile_hook() is None:
        from trn_agent_boot.trn_boot import _ntff_profile_via_ctypes

        ah.set_axon_ntff_profile_hook(
            _ntff_profile_via_ctypes("/opt/axon/libaxon_pjrt.so")
        )

    import concourse.bass_utils as bu

    bu.upload_artifacts = lambda tmpdir: tmpdir


def benchmark(trace_cores=None, **inputs):
    """Run with NTFF tracing; returns (output, BassKernelResults)."""
    _install_ntff_hook()
    in_maps = _prep_inputs(**inputs)
    res = _run(in_maps, trace=True, trace_cores=trace_cores)
    out = np.stack([r["out"].reshape(O, 1) for r in res.results])
    return out, res
